# revision 30
# baseline (speedup 1.0000x reference)
"""DeepTEN encoding kernel for Trainium2 (8 NeuronCores, SPMD data-parallel over batch).

Math (per batch b):
    xf = x[b] viewed (D, N), N = H*W
    dist[n,k] = ||xf[:,n] - c[k]||^2 ;  logits = -scale * dist ;  A = softmax_k(logits)
    E[k,d] = sum_n A[n,k] * (xf[d,n] - c[k,d]) = (A^T X)[k,d] - colsum(A)[k]*c[k,d]

Device decomposition (softmax in (n-partitions, k-free) layout, 2048-n blocks):
    w = -scale (>0), maxs = max(w), wm = w - maxs
    The shifted logit  (wm_k*xsq_n + w_k*csq_k) - 2*w_k*<x_n,c_k>  is built
    entirely in PSUM by the PE:
      * a 64-row block-diagonal "seed" matmul supplies wm_k*xsq_n + w_k*csq_k
        for all 16 n-tiles of a block at once (4 contraction rows per tile:
        xsq_hi, xsq_lo, xsq_hi, ones against wm_hi, wm_hi, wm_lo, wcsq; the
        bf16 hi/lo splits keep the exponent error < ~3e-3)
      * 16 fp8(e4m3) x-tile-stationary matmuls accumulate -2*w_k*<x,c>;
        w1 = -2*w*c is pre-scaled by 64 into fp8 range and the exp rescales
        by 1/64 (ACT computes exp(psum/64) in one fused op)
    P = exp(psum/64) ;  S[n] = sum_k P ;  A = P / S

Aggregation with fp8 xt and the k* residual trick: the softmax is dominated
by k* = argmax(w) (A[n,k*] ~ 1 for most n), so quantizing xt to fp8 with the
full A would accumulate sum_n A*eps errors ~ sqrt(12000)*0.036.  Instead the
device computes R = A - onehot(k*) (by subtracting S from P's k* column
before the 1/S normalization), so the fp8 error scales with sum R^2 ~ 300:
    psum_E[strip(i), 0:D] += sum_n R[n,k]*xt8[n,d]   (4-way col-tiled PE,
    psum_E[strip(i), D]   += sum_n R[n,k]             ones-column colsum)
and the host adds back the exact onehot(k*) term:  E[k*,:] += sum_n x[n,:]
(fp32 on host), colsum[k*] += N.

x is uploaded twice, both fp8 -- (D,N) for the distance matmuls and
pre-transposed (p, gi, d) tiles for the aggregation -- so no on-device
transpose is needed; total HBM read ~2 bytes/elem of x.  mm2 is 4-way
column-tiled across PE col-strips (psum partitions 32j..32j+32), ping-pong
over two PSUM banks -> 8 partial accumulators summed on host.  The softmax
normalization multiply runs on GPSIMD (otherwise idle) to keep DVE off the
critical path.  The mm2s of block j are emitted after the softmax chain of
block j+1 (software pipelining).
"""
import os
import sys
import numpy as np

sys.path.insert(0, "/opt/trn_rl_repo")

import ml_dtypes  # noqa: E402

BF16 = ml_dtypes.bfloat16
FP8 = ml_dtypes.float8_e4m3

B, D, H, W = 32, 128, 128, 128
K = 32
N = H * W            # 16384
NCORES = 8
BPC = B // NCORES    # batches per core
TILN = 128           # n per tile (matmul stationary width)
NTIL = 16            # tiles per block
BLKN = TILN * NTIL   # 2048 n per block
NBLK = N // BLKN     # 8 blocks per batch
W1SCALE = 64.0       # fp8 range scale on w1, undone inside the exp

_CACHE = {}


def _build_module(kstar):
    from contextlib import ExitStack
    import concourse.tile as tile
    from concourse import bacc, mybir

    nc = bacc.Bacc("TRN2", target_bir_lowering=False, debug=False, num_devices=NCORES)
    bf = mybir.dt.bfloat16
    f32 = mybir.dt.float32
    f8 = mybir.dt.float8e4

    x_d = nc.dram_tensor("x", [BPC, D, N], f8, kind="ExternalInput").ap()
    # xt[b, p, gi, d] = x[b, d, gi*128 + p]  (fp8; col D is the ones column)
    xt_d = nc.dram_tensor("xt", [BPC, 128, N // TILN, D + 1], f8, kind="ExternalInput").ap()
    # xsq4[b, i*4+r, j, p]: per-block seed lhsT rows; r in {hi, lo, hi, ones}
    xsq4_d = nc.dram_tensor("xsq4", [BPC, 4 * NTIL, NBLK, 128], bf, kind="ExternalInput").ap()
    # bdg[i*4+r, i*K+k]: block-diagonal seed rhs = 64*[wm_hi, wm_hi, wm_lo, wcsq][k]
    bdg_d = nc.dram_tensor("bdg", [4 * NTIL, NTIL * K], bf, kind="ExternalInput").ap()
    w1_d = nc.dram_tensor("w1", [D, K], f8, kind="ExternalInput").ap()
    oute_d = nc.dram_tensor("out_e", [BPC, 128, 2, D + 1], f32, kind="ExternalOutput").ap()

    with tile.TileContext(nc) as tc, ExitStack() as ctx:
        cpool = ctx.enter_context(tc.tile_pool(name="const", bufs=1))
        xpool = ctx.enter_context(tc.tile_pool(name="xblk", bufs=6))
        xtpool = ctx.enter_context(tc.tile_pool(name="xtblk", bufs=6))
        qpool = ctx.enter_context(tc.tile_pool(name="xsqb", bufs=2))
        ppool = ctx.enter_context(tc.tile_pool(name="pexp", bufs=5))
        npool = ctx.enter_context(tc.tile_pool(name="pnorm", bufs=5))
        vpool = ctx.enter_context(tc.tile_pool(name="small", bufs=4))
        ps_xc = ctx.enter_context(tc.tile_pool(name="ps_xc", bufs=4, space="PSUM"))
        ps_e = ctx.enter_context(tc.tile_pool(name="ps_e", bufs=2, space="PSUM"))

        w1_sb = cpool.tile([D, K], f8)
        nc.sync.dma_start(out=w1_sb[:], in_=w1_d[:, :])
        bdg_sb = cpool.tile([4 * NTIL, NTIL * K], bf)
        nc.sync.dma_start(out=bdg_sb[:], in_=bdg_d[:, :])

        # Software pipeline: mm2s of block j are emitted after the softmax
        # chain of block j+1, so the PE hides the chain latency.
        pending = []  # (b, pn_sb, xt2_sb, hb, i0, i1, finish)
        psum_es = {}
        xsq4_bs = {}
        first_mm2 = {}

        def emit_mm2s(b, pn_sb, xt2_sb, hb, i0, i1, finish):
            pe0, pe1 = psum_es[b]
            ff = first_mm2[b]
            for i in range(i0, i1):
                acc = i % 8              # (bank, strip) accumulator index
                bank = (pe0, pe1)[acc // 4]
                strip = acc % 4
                nc.tensor.matmul(
                    bank[32 * strip : 32 * (strip + 1), :],
                    lhsT=pn_sb[:, K * i : K * (i + 1)],
                    rhs=xt2_sb[:, hb + i, :],
                    start=ff[acc],
                    stop=(finish and i >= NTIL - 8),
                    skip_group_check=True,
                    tile_position=(0, 32 * strip),
                )
                ff[acc] = False
            if finish:
                e_sb = vpool.tile([128, 2, D + 1], f32, tag="e_out")
                nc.scalar.activation(
                    e_sb[:, 0, :], pe0[:], mybir.ActivationFunctionType.Copy
                )
                nc.vector.tensor_copy(e_sb[:, 1, :], pe1[:])
                e_outs.append((b, e_sb))

        def fetch_xsq4(b):
            xsq4_b = qpool.tile(
                [4 * NTIL, NBLK, 128], bf, name=f"xsq4_b{b}", tag="xsqb"
            )
            nc.scalar.dma_start(out=xsq4_b[:], in_=xsq4_d[b])
            xsq4_bs[b] = xsq4_b

        XT_LOOKAHEAD = 3  # pairs (6 blocks) of xt triggers ahead of the EXPs
        xt_tiles = {}
        x_tiles = {}
        e_outs = []

        def fetch_xt_pair(p):
            bp, blkp = divmod(2 * p, NBLK)
            t = xtpool.tile([128, 2 * NTIL, D + 1], f8)
            nc.scalar.dma_start(
                out=t[:], in_=xt_d[bp][:, blkp * NTIL : (blkp + 2) * NTIL, :]
            )
            xt_tiles[p] = t

        def fetch_x_pair(p):
            bp, blkp = divmod(2 * p, NBLK)
            t = xpool.tile([D, 2 * BLKN], f8)
            nc.sync.dma_start(
                out=t[:], in_=x_d[bp][:, blkp * BLKN : (blkp + 2) * BLKN]
            )
            x_tiles[p] = t

        fetch_xsq4(0)
        fetch_x_pair(0)
        fetch_xt_pair(0)
        for p in range(1, XT_LOOKAHEAD):
            fetch_x_pair(p)
            fetch_xt_pair(p)
        for gblk in range(BPC * NBLK):
            b, blk = divmod(gblk, NBLK)
            if blk == 0:
                if b + 1 < BPC:
                    fetch_xsq4(b + 1)  # prefetch next batch's seed rows
                psum_es[b] = (
                    ps_e.tile([128, D + 1], f32, tag="pe0", name=f"psum_e0_b{b}"),
                    ps_e.tile([128, D + 1], f32, tag="pe1", name=f"psum_e1_b{b}"),
                )
                first_mm2[b] = [True] * 8
            if blk % 2 == 0:
                pf = gblk // 2 + XT_LOOKAHEAD
                if pf < BPC * NBLK // 2:
                    fetch_x_pair(pf)
                    fetch_xt_pair(pf)
                x2_sb = x_tiles[gblk // 2]
                xt2_sb = xt_tiles[gblk // 2]
            if gblk == BPC * NBLK - 2:
                # all fetch doorbells are issued by now: flush finished
                # batches' outputs so the epilogue only pays the last batch
                while e_outs:
                    b_, e_sb_ = e_outs.pop(0)
                    nc.sync.dma_start(out=oute_d[b_], in_=e_sb_[:])
            hb = (blk % 2) * NTIL
            psum_xc = ps_xc.tile([128, NTIL * K], f32)
            nc.tensor.matmul(
                psum_xc[:],
                lhsT=xsq4_bs[b][:, blk, :],
                rhs=bdg_sb[:],
                start=True,
                stop=False,
                skip_group_check=True,
            )
            for i in range(NTIL):
                nc.tensor.matmul(
                    psum_xc[:, K * i : K * (i + 1)],
                    lhsT=x2_sb[:, (hb + i) * TILN : (hb + i + 1) * TILN],
                    rhs=w1_sb[:, :],
                    start=False,
                    stop=True,
                    skip_group_check=True,
                )

            # Softmax chain: exp (ACT) -> rowsum (DVE) -> subtract S from the
            # k* column (DVE, makes R = A - onehot(k*) after normalize) ->
            # 1/S (DVE) -> P*Sinv (GPSIMD).
            # The last blocks run in halves/quarters so the pipeline drain at
            # the end of the kernel pays smaller chain stages.
            tail = BPC * NBLK - 1 - gblk
            nh = 4 if tail == 0 else (2 if tail == 1 else 1)
            ht = NTIL // nh
            p_sb = ppool.tile([128, NTIL * K], bf, tag="pexp")
            s_sb = vpool.tile([128, NTIL], f32, tag="s")
            sinv_sb = vpool.tile([128, NTIL], f32, tag="sinv")
            pn_sb = npool.tile([128, NTIL * K], bf, tag="pn")
            for h in range(nh):
                tsl = slice(h * ht, (h + 1) * ht)
                csl = slice(h * ht * K, (h + 1) * ht * K)
                nc.scalar.activation(
                    p_sb[:, csl],
                    psum_xc[:, csl],
                    mybir.ActivationFunctionType.Exp,
                    scale=1.0 / W1SCALE,
                )
                p3 = p_sb[:, csl].rearrange("p (i k) -> p i k", k=K)
                nc.vector.reduce_sum(s_sb[:, tsl], p3, axis=mybir.AxisListType.X)
                pk = p_sb[:, h * ht * K + kstar : (h + 1) * ht * K : K]  # [128, ht]
                nc.vector.tensor_tensor(
                    pk,
                    pk,
                    s_sb[:, tsl],
                    op=mybir.AluOpType.subtract,
                )
                nc.vector.reciprocal_approx_fast(out=sinv_sb[:, tsl], in_=s_sb[:, tsl])
                nc.gpsimd.tensor_tensor(
                    pn_sb[:, csl].rearrange("p (i k) -> p i k", k=K),
                    p3,
                    sinv_sb[:, tsl].broadcast_to([128, ht, K]),
                    op=mybir.AluOpType.mult,
                )
                pending.append(
                    (b, pn_sb, xt2_sb, hb, h * ht, (h + 1) * ht,
                     blk == NBLK - 1 and h == nh - 1)
                )
                if len(pending) > 2:
                    emit_mm2s(*pending.pop(0))

        while pending:
            emit_mm2s(*pending.pop(0))
        for b_, e_sb_ in e_outs:
            nc.sync.dma_start(out=oute_d[b_], in_=e_sb_[:])

    nc.compile()
    return nc


def _get_module(kstar):
    if kstar not in _CACHE:
        _CACHE[kstar] = _build_module(kstar)
    return _CACHE[kstar]


def _host_prep(x, codewords, scale):
    x = np.asarray(x, dtype=np.float32)
    c = np.asarray(codewords, dtype=np.float32)
    s = np.asarray(scale, dtype=np.float32)

    w = -s                           # (K,) in (0, 1)
    maxs = float(w.max())
    kstar = int(np.argmax(w))
    w1 = (-2.0 * W1SCALE * (w[:, None] * c)).T.astype(FP8)  # (D, K) fp8, x64
    wm = w - maxs                                           # (K,) <= 0
    wm_hi = wm.astype(BF16).astype(np.float32)
    wm_lo = wm - wm_hi
    wcsq = w * (c * c).sum(axis=1)                          # (K,)

    xf = x.reshape(B, D, N)
    sum_x = xf.astype(np.float64).sum(axis=2).astype(np.float32)  # (B, D)
    xsq = np.einsum("bdn,bdn->bn", xf, xf)                  # (B, N) fp32
    xsq_hi = xsq.astype(BF16)
    xsq_lo = (xsq - xsq_hi.astype(np.float32)).astype(BF16)
    # xsq4[b, i*4+r, j, p]: r = 0..3 -> (xsq_hi, xsq_lo, xsq_hi, 1); the seed
    # value at psum[p, (i,k)] is sum_r xsq4[b,i*4+r,j,p] * bdg[i*4+r, i*K+k]
    hi_r = np.ascontiguousarray(
        xsq_hi.reshape(B, NBLK, NTIL, 128).transpose(0, 2, 1, 3)
    )                                                       # (B, 16, NBLK, 128)
    lo_r = np.ascontiguousarray(
        xsq_lo.reshape(B, NBLK, NTIL, 128).transpose(0, 2, 1, 3)
    )
    xsq4 = np.empty((B, NTIL, 4, NBLK, 128), dtype=BF16)
    xsq4[:, :, 0] = hi_r
    xsq4[:, :, 1] = lo_r
    xsq4[:, :, 2] = hi_r
    xsq4[:, :, 3] = 1.0
    xsq4 = xsq4.reshape(B, 4 * NTIL, NBLK, 128)

    bdg = np.zeros((4 * NTIL, NTIL * K), dtype=BF16)
    rows = np.stack(
        [W1SCALE * wm_hi, W1SCALE * wm_lo, W1SCALE * wcsq]
    ).astype(BF16)                                          # (3, K)
    for i in range(NTIL):
        bdg[i * 4 + 0, i * K : (i + 1) * K] = rows[0]
        bdg[i * 4 + 1, i * K : (i + 1) * K] = rows[0]
        bdg[i * 4 + 2, i * K : (i + 1) * K] = rows[1]
        bdg[i * 4 + 3, i * K : (i + 1) * K] = rows[2]

    x8 = xf.astype(FP8)                                     # (B, D, N)
    # xt[b, p, gi, d] = xf[b, d, gi*128 + p];  xt[..., D] = 1.0 (fused colsum column)
    xt = np.ones((B, N // TILN, TILN, D + 1), dtype=FP8)
    xt[:, :, :, :D] = xf.transpose(0, 2, 1).reshape(B, N // TILN, TILN, D).astype(FP8)
    xt = np.ascontiguousarray(xt.transpose(0, 2, 1, 3))     # (B, 128, N/128, D+1)
    return x8, xt, xsq4, bdg, w1, sum_x, kstar


def make_in_maps(x, codewords, scale):
    x8, xt, xsq4, bdg, w1, sum_x, kstar = _host_prep(x, codewords, scale)
    in_maps = []
    for ci in range(NCORES):
        sl = slice(BPC * ci, BPC * (ci + 1))
        in_maps.append(
            {
                "x": np.ascontiguousarray(x8[sl]),
                "xt": np.ascontiguousarray(xt[sl]),
                "xsq4": np.ascontiguousarray(xsq4[sl]),
                "bdg": bdg,
                "w1": w1,
            }
        )
    return in_maps, sum_x, kstar


def finish_output(results, codewords, sum_x, kstar):
    c = np.asarray(codewords, dtype=np.float32)
    out = np.zeros((B, K * D), dtype=np.float32)
    for ci, r in enumerate(results):
        for bb in range(BPC):
            # [128, 2, D+1] -> 8 partial accumulators of [K, D+1]
            e_parts = (
                r["out_e"][bb].reshape(4, K, 2, D + 1).sum(axis=(0, 2))
            )
            e = e_parts[:, :D]
            cols = e_parts[:, D].copy()
            bg = BPC * ci + bb
            e[kstar] += sum_x[bg]
            cols[kstar] += float(N)
            out[bg] = (e - cols[:, None] * c).reshape(-1)
    return out


def kernel(x, codewords, scale):
    from concourse.bass_utils import run_bass_kernel_spmd
    from concourse.bass_interp import get_hw_module

    in_maps, sum_x, kstar = make_in_maps(x, codewords, scale)
    nc = _get_module(kstar)

    old_m = nc.m
    nc.m = get_hw_module(nc.m)
    try:
        res = run_bass_kernel_spmd(nc, in_maps, core_ids=list(range(NCORES)))
    finally:
        nc.m = old_m
    return finish_output(res.results, codewords, sum_x, kstar)
